# revision 2
# baseline (speedup 1.0000x reference)
"""Embedding lookup kernel for Trainium2 (8 NeuronCores, SPMD).

Strategy: token-parallel gather (an embedding lookup IS a row gather:
out[b, s, :] = weight[x[b, s], :]).

- Flatten x [2, 4096] -> [8192] tokens; each of the 8 cores handles 1024
  contiguous tokens. Each core gets the FULL weight table in its DRAM.
- Per core (raw Bacc program, no Tile framework overhead):
    1. One HWDGE DMA (issued from the Scalar engine, which exits the
       framework preamble earlier than Sync) loads the 1024 indices as
       [128, 64] int16 into SBUF. Layout required by dma_gather: index i
       at partition i%16, column i//16, replicated across the 8 groups of
       16 partitions (one replica per GpSimd Q7 core). VOCAB=32000 <
       2^15, so int16 indices are exact; the host prepares this tile.
    2. While that DMA's ~2us HBM round trip elapses, a dummy warmup
       dma_gather (16 indices from a memset-zero tile) runs on the Pool
       engine so the mlp-library ucode + first-op overheads are paid
       inside the idx-load latency window.
    3. ONE SWDGE dma_gather fetches all 1024 rows (512 KiB) of the
       embedding table into an SBUF tile g[128, 8, 128] f32 with row i at
       [i%128, i//128, :]. Descriptor generation is ~1us fixed +
       0.34ns/descriptor -- vs the 8x ~1.1us serialized indirect-DMA ops
       this replaces (the old kernel's dominant cost).
    4. One HWDGE DMA stores g to DRAM out[128, 8, 128]. No final
       completion wait: the NEFF epilogue's engine drains block until the
       HWDGE queues are empty (verified bit-exact on HW by the previous
       kernel, same mechanism).
- Host: out[p, c, :] holds token c*128+p, so transpose(1,0,2) recovers
  token-major [1024, 128]; concatenate the 8 per-core outputs.

No collectives.
"""

import contextlib

import numpy as np

import concourse.bass as bass
from concourse import bacc, mybir
from concourse.bass_utils import run_bass_kernel_spmd

N_CORES = 8
B, S = 2, 4096
VOCAB, DIM = 32000, 128
P = 128
TOKENS = B * S                      # 8192
TPC = TOKENS // N_CORES             # 1024 tokens per core
CB = TPC // P                       # 8 column blocks of 128 tokens
IDX_COLS = TPC // 16                # 64 int16 per idx partition


def build_nc():
    # Skip the Bass-constructor entry barrier (gates the first DMA behind
    # all engines' init); restore the method right after construction.
    orig_barrier = bass.Bass.all_engine_barrier
    bass.Bass.all_engine_barrier = lambda self, *a, **k: None
    try:
        nc = bacc.Bacc(None, target_bir_lowering=False)
    finally:
        bass.Bass.all_engine_barrier = orig_barrier

    x = nc.dram_tensor("x", [P, IDX_COLS], mybir.dt.int16, kind="ExternalInput")
    w = nc.dram_tensor("weight", [VOCAB, DIM], mybir.dt.float32, kind="ExternalInput")
    out = nc.dram_tensor("out", [P, CB, DIM], mybir.dt.float32, kind="ExternalOutput")

    with contextlib.ExitStack() as ctx:
        idx_tile = ctx.enter_context(
            nc.sbuf_tensor("idx_tile", [P, IDX_COLS], mybir.dt.int16)
        )
        g = ctx.enter_context(nc.sbuf_tensor("g", [P, CB, DIM], mybir.dt.float32))
        dummy_idx = ctx.enter_context(
            nc.sbuf_tensor("dummy_idx", [P, 1], mybir.dt.int16)
        )
        scratch = ctx.enter_context(
            nc.sbuf_tensor("scratch", [P, 1, DIM], mybir.dt.float32)
        )
        s_idx = ctx.enter_context(nc.semaphore("s_idx"))
        s_ms = ctx.enter_context(nc.semaphore("s_ms"))
        s_warm = ctx.enter_context(nc.semaphore("s_warm"))
        s_g = ctx.enter_context(nc.semaphore("s_g"))
        s_out = ctx.enter_context(nc.semaphore("s_out"))

        nc.scalar.dma_start(idx_tile[:], x[:]).then_inc(s_idx, 16)

        # Warmup gather (library ucode + first-op cost), hidden inside the
        # idx-DMA latency window.
        nc.gpsimd.memset(dummy_idx[:], 0).then_inc(s_ms, 1)
        nc.gpsimd.wait_ge(s_ms, 1)
        nc.gpsimd.dma_gather(
            scratch[:], w[:], dummy_idx[:], 16, 16, DIM
        ).then_inc(s_warm, 16)

        nc.gpsimd.wait_ge(s_idx, 16)
        nc.gpsimd.dma_gather(g[:], w[:], idx_tile[:], TPC, TPC, DIM).then_inc(
            s_g, 16
        )

        nc.sync.wait_ge(s_g, 16)
        nc.sync.dma_start(out[:], g[:]).then_inc(s_out, 16)
    nc.compile()
    return nc


_NC_CACHE = None


def _pack_idx(idx_1d: np.ndarray) -> np.ndarray:
    """[TPC] int -> [128, 64] int16 dma_gather index tile."""
    wrapped = idx_1d.astype(np.int16).reshape(IDX_COLS, 16).T  # [16, 64]
    return np.ascontiguousarray(np.tile(wrapped, (P // 16, 1)))


def kernel(x: np.ndarray, weight: np.ndarray, **run_kwargs):
    global _NC_CACHE
    if _NC_CACHE is None:
        _NC_CACHE = build_nc()
    nc = _NC_CACHE

    x_flat = np.asarray(x).reshape(-1)
    w = np.ascontiguousarray(np.asarray(weight, dtype=np.float32))

    in_maps = [
        {
            "x": _pack_idx(x_flat[c * TPC : (c + 1) * TPC]),
            "weight": w,
        }
        for c in range(N_CORES)
    ]
    res = run_bass_kernel_spmd(nc, in_maps, core_ids=list(range(N_CORES)), **run_kwargs)
    # out [128, 8, 128]: token c*128+p at [p, c, :] -> token-major [1024, 128]
    parts = [
        res.results[c]["out"].transpose(1, 0, 2).reshape(TPC, DIM)
        for c in range(N_CORES)
    ]
    full = np.concatenate(parts, axis=0).reshape(B, S, DIM)
    if run_kwargs:
        return full, res
    return full


# revision 6
# speedup vs baseline: 1.3305x; 1.3305x over previous
"""Embedding lookup kernel for Trainium2 (8 NeuronCores, SPMD).

Strategy: token-parallel gather (an embedding lookup IS a row gather:
out[b, s, :] = weight[x[b, s], :]).

- Flatten x [2, 4096] -> [8192] tokens; each of the 8 cores handles 1024
  contiguous tokens. Each core gets the FULL weight table in its DRAM.
- Per core (raw Bacc program, no Tile framework overhead; the Bass entry
  all-engine barrier is skipped):
    1. One HWDGE DMA issued from the Scalar engine (exits the framework
       preamble ~0.8us before Sync) loads the 1024 indices as [128, 8]
       int32 into SBUF (partition p holds tokens p*8 .. p*8+7).
       Completion is HBM-round-trip-bound (~2.4us).
    2. While that latency elapses, a dummy warmup indirect DMA (indices
       from a memset-zero tile) runs on the Pool engine so the first real
       gather executes at steady-state cost.
    3. 8 SWDGE indirect DMAs (one per token column j; one index per
       partition is the HW limit, verified: a [128, 8] offset AP makes
       the HW emit one 4KB-contiguous descriptor per partition). Each
       gathers 128 rows DIRECTLY INTO DRAM out[:, j*128:(j+1)*128]
       (DRAM->DRAM), skipping the SBUF bounce + HWDGE store of the
       previous version. SWDGE descriptor generation (~8.5ns/row,
       Q7-serial) is the dominant cost; the 512 KiB of gather traffic
       drains under it.
    4. Only the LAST gather carries a completion semaphore: SWDGE ring
       FIFO ordering per SDMA engine means op7's sem implies ops 0-6
       drained. The Pool engine waits for it before the NEFF epilogue.
- dynamic_dma_scratch_size=65536 (4x default): the default descriptor
  ring holds exactly 8x128 descs, so later ops stall on ring reclaim.
- out [128, 1024] f32 reshapes host-side to [1024, 128] (token p*8+j at
  partition p, col-block j). Host concatenates the 8 per-core outputs.

Set EMB_VARIANT=sbuf to fall back to the SBUF-bounce + per-block-store
variant (same as the 23.7us baseline but with the scalar-idx/scratch/
semaphore trims).

No collectives. (A dma_gather variant was measured SLOWER: the mlp
Q7-library load costs ~8.7us in-kernel and desc-gen runs at the same
~8.5ns/row.)
"""

import contextlib
import os

import numpy as np

import concourse.bass as bass
from concourse import bacc, mybir
from concourse.bass_utils import run_bass_kernel_spmd

N_CORES = 8
B, S = 2, 4096
VOCAB, DIM = 32000, 128
P = 128
TOKENS = B * S                      # 8192
TPC = TOKENS // N_CORES             # 1024 tokens per core
TPP = TPC // P                      # 8 tokens per partition

VARIANT = os.environ.get("EMB_VARIANT", "dram")


def _indirect_gather(gp, out_ap, in_ap, offset_ap):
    """indirect_dma_start clone without the SBUF-dest restriction: allows
    DRAM->DRAM row gather (out[p, :] = in_[offset[p], :] per partition)."""
    nc_bass = gp.bass
    out_l = gp.lower_ap_dma(out_ap, for_indirect_dma=True)
    in_l = gp.lower_ap_dma(in_ap, for_indirect_dma=True)
    assert len(in_l) == 1 and len(out_l) == 1
    off_l = gp.lower_ap_dma(offset_ap)
    assert len(off_l) == 1
    in_l.append(off_l[0])

    ap_shape = in_ap.shape
    coef = 1
    for i in range(1, len(ap_shape)):
        coef *= ap_shape[i]
    in_l[0].dynamic_ap_info = mybir.DynamicAccessPatternInfo(
        c=0,
        actual_ap=out_ap.ap,
        indirect_dim_max_index=ap_shape[0],
        offset_expr=[
            mybir.DynamicAccessPatternOffsetExpr(
                coef=coef,
                aff_expr=mybir.DynamicAccessPatternOffsetExprAffExpr(
                    kind="IndirectArgId",
                    arg_id=1,
                ),
            )
        ],
    )
    return gp.add_instruction(
        mybir.InstDMACopy(
            name=nc_bass.get_next_instruction_name(),
            queue="qPoolDynamic",
            mode="Copy",
            ins=in_l,
            outs=out_l,
            oob_is_err=True,
            cce_op=mybir.AluOpType.bypass,
        )
    )


def build_nc():
    # Skip the Bass-constructor entry barrier (gates the first DMA behind
    # all engines' init); restore the method right after construction.
    orig_barrier = bass.Bass.all_engine_barrier
    bass.Bass.all_engine_barrier = lambda self, *a, **k: None
    try:
        nc = bacc.Bacc(
            None,
            target_bir_lowering=False,
            dynamic_dma_scratch_size=int(
                os.environ.get("EMB_SCRATCH", "16384")
            ),
        )
    finally:
        bass.Bass.all_engine_barrier = orig_barrier

    x = nc.dram_tensor("x", [P, TPP], mybir.dt.int32, kind="ExternalInput")
    w = nc.dram_tensor("weight", [VOCAB, DIM], mybir.dt.float32, kind="ExternalInput")
    out = nc.dram_tensor("out", [P, TPC], mybir.dt.float32, kind="ExternalOutput")
    waste = nc.dram_tensor("waste", [P, DIM], mybir.dt.float32, kind="Internal")

    with contextlib.ExitStack() as ctx:
        idx_tile = ctx.enter_context(
            nc.sbuf_tensor("idx_tile", [P, TPP], mybir.dt.int32)
        )
        dummy_idx = ctx.enter_context(
            nc.sbuf_tensor("dummy_idx", [P, 1], mybir.dt.int32)
        )
        s_idx = ctx.enter_context(nc.semaphore("s_idx"))
        s_ms = ctx.enter_context(nc.semaphore("s_ms"))
        s_warm = ctx.enter_context(nc.semaphore("s_warm"))
        s_g = ctx.enter_context(nc.semaphore("s_g"))

        nc.scalar.dma_start(idx_tile[:], x[:]).then_inc(s_idx, 16)

        # Warmup gather, hidden inside the idx-DMA latency window.
        nc.gpsimd.memset(dummy_idx[:], 0).then_inc(s_ms, 1)
        nc.gpsimd.wait_ge(s_ms, 1)

        if VARIANT == "dram":
            _indirect_gather(nc.gpsimd, waste[:], w[:], dummy_idx[:]).then_inc(
                s_warm, 16
            )
            nc.gpsimd.wait_ge(s_idx, 16)
            for j in range(TPP):
                _indirect_gather(
                    nc.gpsimd,
                    out[:, j * DIM : (j + 1) * DIM],
                    w[:],
                    idx_tile[:, j : j + 1],
                ).then_inc(s_g, 16)
            nc.gpsimd.wait_ge(s_g, 16 * TPP)
        else:  # sbuf-bounce variant
            g = ctx.enter_context(
                nc.sbuf_tensor("g", [P, TPC], mybir.dt.float32)
            )
            scratch = ctx.enter_context(
                nc.sbuf_tensor("scratch", [P, DIM], mybir.dt.float32)
            )
            s_out = ctx.enter_context(nc.semaphore("s_out"))
            nc.gpsimd.indirect_dma_start(
                out=scratch[:],
                out_offset=None,
                in_=w[:],
                in_offset=bass.IndirectOffsetOnAxis(ap=dummy_idx[:], axis=0),
            ).then_inc(s_warm, 16)
            nc.gpsimd.wait_ge(s_idx, 16)
            for j in range(TPP):
                nc.gpsimd.indirect_dma_start(
                    out=g[:, j * DIM : (j + 1) * DIM],
                    out_offset=None,
                    in_=w[:],
                    in_offset=bass.IndirectOffsetOnAxis(
                        ap=idx_tile[:, j : j + 1], axis=0
                    ),
                ).then_inc(s_g, 16)
            for j in range(TPP):
                nc.sync.wait_ge(s_g, 16 * (j + 1))
                nc.sync.dma_start(
                    out[:, j * DIM : (j + 1) * DIM], g[:, j * DIM : (j + 1) * DIM]
                ).then_inc(s_out, 16)
    nc.compile()
    return nc


_NC_CACHE = None


def kernel(x: np.ndarray, weight: np.ndarray, **run_kwargs):
    global _NC_CACHE
    if _NC_CACHE is None:
        _NC_CACHE = build_nc()
    nc = _NC_CACHE

    x_flat = np.asarray(x).reshape(-1).astype(np.int32)
    w = np.ascontiguousarray(np.asarray(weight, dtype=np.float32))

    in_maps = [
        {
            "x": np.ascontiguousarray(x_flat[c * TPC : (c + 1) * TPC].reshape(P, TPP)),
            "weight": w,
        }
        for c in range(N_CORES)
    ]
    res = run_bass_kernel_spmd(nc, in_maps, core_ids=list(range(N_CORES)), **run_kwargs)
    # out [128, 1024] -> [1024, 128]: token p*TPP+j lives at [p, j*DIM:(j+1)*DIM]
    parts = [res.results[c]["out"].reshape(TPC, DIM) for c in range(N_CORES)]
    full = np.concatenate(parts, axis=0).reshape(B, S, DIM)
    if run_kwargs:
        return full, res
    return full


# revision 7
# speedup vs baseline: 1.3430x; 1.0095x over previous
"""Embedding lookup kernel for Trainium2 (8 NeuronCores, SPMD).

Strategy: token-parallel gather (an embedding lookup IS a row gather:
out[b, s, :] = weight[x[b, s], :]).

- Flatten x [2, 4096] -> [8192] tokens; each of the 8 cores handles 1024
  contiguous tokens. Each core gets the FULL weight table in its DRAM.
- Per core (raw Bacc program, no Tile framework overhead; the Bass entry
  all-engine barrier is skipped):
    1. One HWDGE DMA issued from the Scalar engine (exits the framework
       preamble ~0.9us before Sync) loads the 1024 indices as [128, 8]
       int32 into SBUF (partition p holds tokens p*8 .. p*8+7).
       Completion is HBM-round-trip-bound (~2.4us).
    2. While that latency elapses, a dummy 16-row warmup indirect DMA
       (zero indices from a memset tile; no semaphore between them --
       memset runs on the same Q7 cluster, so program order suffices)
       runs on the Pool engine so the first real gather executes at
       steady-state cost. 16 rows (8KB) instead of 128 keeps the warmup
       SDMA traffic from delaying the idx DMA completion.
    3. 8 SWDGE indirect DMAs (one per token column j; one index per
       partition is a HW limit -- verified: a [128, 8] offset AP makes
       the HW emit one 4KB-contiguous descriptor per partition, and
       DRAM->DRAM gathers wedge the device). Each op costs ~1.41us on
       the Pool engine (~1.10us descriptor generation for 128 rows +
       ~0.31us fixed post-op overhead); the 512 KiB of gather traffic
       drains under it.
    4. As each gather's completion semaphore fires (single accumulating
       semaphore), an HWDGE DMA stores that column block to DRAM
       out[:, j*128:(j+1)*128], overlapping the remaining gathers. No
       final completion wait: the NEFF epilogue's engine drains block
       until the HWDGE queues are empty (verified bit-exact on HW).
- dynamic_dma_scratch_size=65536 (4x default) so the SWDGE descriptor
  ring never stalls on reclaim (default holds exactly 8x128 descs).
- out [128, 1024] f32 reshapes host-side to [1024, 128] (token p*8+j at
  partition p, col-block j). Host concatenates the 8 per-core outputs.

No collectives. Rejected alternatives (all measured or compiler-blocked):
dma_gather (mlp Q7-library load costs ~8.7us in-kernel; same ~8.5ns/row
desc-gen), DRAM->DRAM indirect (NRT_EXEC_UNIT_UNRECOVERABLE on HW),
multi-index-per-partition offset APs (HW emits contiguous-block
descriptors instead), one-hot matmul (compute-bound, ~23us+ at vocab/8
per core).
"""

import contextlib
import os

import numpy as np

import concourse.bass as bass
from concourse import bacc, mybir
from concourse.bass_utils import run_bass_kernel_spmd

N_CORES = 8
B, S = 2, 4096
VOCAB, DIM = 32000, 128
P = 128
TOKENS = B * S                      # 8192
TPC = TOKENS // N_CORES             # 1024 tokens per core
TPP = TPC // P                      # 8 tokens per partition

WARM_ROWS = 16


def build_nc():
    # Skip the Bass-constructor entry barrier (gates the first DMA behind
    # all engines' init); restore the method right after construction.
    orig_barrier = bass.Bass.all_engine_barrier
    bass.Bass.all_engine_barrier = lambda self, *a, **k: None
    try:
        nc = bacc.Bacc(
            None,
            target_bir_lowering=False,
            dynamic_dma_scratch_size=int(os.environ.get("EMB_SCRATCH", "65536")),
        )
    finally:
        bass.Bass.all_engine_barrier = orig_barrier

    x = nc.dram_tensor("x", [P, TPP], mybir.dt.int32, kind="ExternalInput")
    w = nc.dram_tensor("weight", [VOCAB, DIM], mybir.dt.float32, kind="ExternalInput")
    out = nc.dram_tensor("out", [P, TPC], mybir.dt.float32, kind="ExternalOutput")

    with contextlib.ExitStack() as ctx:
        idx_tile = ctx.enter_context(
            nc.sbuf_tensor("idx_tile", [P, TPP], mybir.dt.int32)
        )
        g = ctx.enter_context(nc.sbuf_tensor("g", [P, TPC], mybir.dt.float32))
        dummy_idx = ctx.enter_context(
            nc.sbuf_tensor("dummy_idx", [P, 1], mybir.dt.int32)
        )
        scratch = ctx.enter_context(
            nc.sbuf_tensor("scratch", [P, DIM], mybir.dt.float32)
        )
        s_idx = ctx.enter_context(nc.semaphore("s_idx"))
        s_warm = ctx.enter_context(nc.semaphore("s_warm"))
        s_g = ctx.enter_context(nc.semaphore("s_g"))
        s_out = ctx.enter_context(nc.semaphore("s_out"))

        nc.scalar.dma_start(idx_tile[:], x[:]).then_inc(s_idx, 16)

        # Warmup gather, hidden inside the idx-DMA latency window. The
        # memset and the gather's offset read both execute on the Q7
        # cluster, so engine program order makes the write visible.
        nc.gpsimd.memset(dummy_idx[:], 0)
        nc.gpsimd.indirect_dma_start(
            out=scratch[0:WARM_ROWS, :],
            out_offset=None,
            in_=w[:],
            in_offset=bass.IndirectOffsetOnAxis(
                ap=dummy_idx[0:WARM_ROWS, :], axis=0
            ),
        ).then_inc(s_warm, 16)

        nc.gpsimd.wait_ge(s_idx, 16)
        for j in range(TPP):
            nc.gpsimd.indirect_dma_start(
                out=g[:, j * DIM : (j + 1) * DIM],
                out_offset=None,
                in_=w[:],
                in_offset=bass.IndirectOffsetOnAxis(ap=idx_tile[:, j : j + 1], axis=0),
            ).then_inc(s_g, 16)
        for j in range(TPP):
            nc.sync.wait_ge(s_g, 16 * (j + 1))
            nc.sync.dma_start(
                out[:, j * DIM : (j + 1) * DIM], g[:, j * DIM : (j + 1) * DIM]
            ).then_inc(s_out, 16)
    nc.compile()
    return nc


_NC_CACHE = None


def kernel(x: np.ndarray, weight: np.ndarray, **run_kwargs):
    global _NC_CACHE
    if _NC_CACHE is None:
        _NC_CACHE = build_nc()
    nc = _NC_CACHE

    x_flat = np.asarray(x).reshape(-1).astype(np.int32)
    w = np.ascontiguousarray(np.asarray(weight, dtype=np.float32))

    in_maps = [
        {
            "x": np.ascontiguousarray(x_flat[c * TPC : (c + 1) * TPC].reshape(P, TPP)),
            "weight": w,
        }
        for c in range(N_CORES)
    ]
    res = run_bass_kernel_spmd(nc, in_maps, core_ids=list(range(N_CORES)), **run_kwargs)
    # out [128, 1024] -> [1024, 128]: token p*TPP+j lives at [p, j*DIM:(j+1)*DIM]
    parts = [res.results[c]["out"].reshape(TPC, DIM) for c in range(N_CORES)]
    full = np.concatenate(parts, axis=0).reshape(B, S, DIM)
    if run_kwargs:
        return full, res
    return full


# revision 10
# speedup vs baseline: 1.3601x; 1.0127x over previous
"""Embedding lookup kernel for Trainium2 (8 NeuronCores, SPMD).

Strategy: token-parallel gather (an embedding lookup IS a row gather:
out[b, s, :] = weight[x[b, s], :]).

- Flatten x [2, 4096] -> [8192] tokens; each of the 8 cores handles 1024
  contiguous tokens. Each core gets the FULL weight table in its DRAM.
- Per core (raw Bacc program, no Tile framework overhead; the Bass entry
  all-engine barrier is skipped):
    1. One HWDGE DMA issued from the Scalar engine (exits the framework
       preamble ~0.9us before Sync) loads the 1024 indices as [128, 8]
       int32 into SBUF (partition p holds tokens p*8 .. p*8+7).
       Completion is HBM-round-trip-bound (~2.3us).
    2. While that latency elapses, a dummy 16-row warmup indirect DMA
       (zero indices from a memset tile; no semaphore between them --
       memset runs on the same Q7 cluster, so program order suffices)
       runs on the Pool engine, absorbing the first-SWDGE-op ring-setup
       cost (~0.8us) so the real gathers run at steady state. 16 rows
       (8KB) keeps warmup SDMA traffic from delaying the idx DMA.
    3. 8 SWDGE indirect DMAs, one per token column j. One index per
       partition per op is a hard HW behavior: the DGE consumes ONE
       offset per partition and copies the dest partition's free run
       from it (verified: a [128, 8] offset AP yields one 4KB descriptor
       per partition reading idx[p,0] only; a strided dest that forces 8
       descs/partition scrambles data; DRAM->DRAM dest wedges the
       device, NRT_EXEC_UNIT_UNRECOVERABLE). Per-op cost is ~1.41us on
       the Pool engine (~1.03us fixed + ~0.5ns/row + ~0.31us post-op
       gap); the 512 KiB of gather traffic drains underneath.
    4. As each gather's completion fires on one accumulating semaphore,
       an HWDGE DMA stores that column block to DRAM
       out[:, j*128:(j+1)*128], overlapping the remaining gathers. No
       final completion wait: the NEFF epilogue's engine drains block
       until the HWDGE queues are empty (verified bit-exact on HW).
- dynamic_dma_scratch_size=65536 (4x default): the default SWDGE
  descriptor ring holds exactly 8x128 descs, so reclaim could stall the
  op train.
- out [128, 1024] f32 reshapes host-side to [1024, 128] (token p*8+j at
  partition p, col-block j). Host concatenates the 8 per-core outputs.

No collectives. Measured 23.2us exec (neuron-profile), bit-exact vs the
one-hot matmul reference. Rejected alternatives (all measured):
dma_gather batches 1024 rows in one op but its mlp Q7-library load costs
~8.7us in-kernel and its ucode runs at 8.4ns/row (31.1us total); one-hot
matmul is compute-bound (~23us+ at vocab/8 per core); SBUF-resident
table + ap_gather is Q7-throughput-bound plus the same library tax.
"""

import contextlib

import numpy as np

import concourse.bass as bass
from concourse import bacc, mybir
from concourse.bass_utils import run_bass_kernel_spmd

N_CORES = 8
B, S = 2, 4096
VOCAB, DIM = 32000, 128
P = 128
TOKENS = B * S                      # 8192
TPC = TOKENS // N_CORES             # 1024 tokens per core
TPP = TPC // P                      # 8 tokens per partition

WARM_ROWS = 16


def build_nc():
    # Skip the Bass-constructor entry barrier (gates the first DMA behind
    # all engines' init); restore the method right after construction.
    orig_barrier = bass.Bass.all_engine_barrier
    bass.Bass.all_engine_barrier = lambda self, *a, **k: None
    try:
        nc = bacc.Bacc(
            None, target_bir_lowering=False, dynamic_dma_scratch_size=65536
        )
    finally:
        bass.Bass.all_engine_barrier = orig_barrier

    x = nc.dram_tensor("x", [P, TPP], mybir.dt.int32, kind="ExternalInput")
    w = nc.dram_tensor("weight", [VOCAB, DIM], mybir.dt.float32, kind="ExternalInput")
    out = nc.dram_tensor("out", [P, TPC], mybir.dt.float32, kind="ExternalOutput")

    with contextlib.ExitStack() as ctx:
        idx_tile = ctx.enter_context(
            nc.sbuf_tensor("idx_tile", [P, TPP], mybir.dt.int32)
        )
        g = ctx.enter_context(nc.sbuf_tensor("g", [P, TPC], mybir.dt.float32))
        dummy_idx = ctx.enter_context(
            nc.sbuf_tensor("dummy_idx", [P, 1], mybir.dt.int32)
        )
        scratch = ctx.enter_context(
            nc.sbuf_tensor("scratch", [P, DIM], mybir.dt.float32)
        )
        s_idx = ctx.enter_context(nc.semaphore("s_idx"))
        s_warm = ctx.enter_context(nc.semaphore("s_warm"))
        s_g = ctx.enter_context(nc.semaphore("s_g"))
        s_out = ctx.enter_context(nc.semaphore("s_out"))

        nc.scalar.dma_start(idx_tile[:], x[:]).then_inc(s_idx, 16)

        # Warmup gather, hidden inside the idx-DMA latency window.
        nc.gpsimd.memset(dummy_idx[:], 0)
        nc.gpsimd.indirect_dma_start(
            out=scratch[0:WARM_ROWS, :],
            out_offset=None,
            in_=w[:],
            in_offset=bass.IndirectOffsetOnAxis(
                ap=dummy_idx[0:WARM_ROWS, :], axis=0
            ),
        ).then_inc(s_warm, 16)

        nc.gpsimd.wait_ge(s_idx, 16)
        for j in range(TPP):
            nc.gpsimd.indirect_dma_start(
                out=g[:, j * DIM : (j + 1) * DIM],
                out_offset=None,
                in_=w[:],
                in_offset=bass.IndirectOffsetOnAxis(ap=idx_tile[:, j : j + 1], axis=0),
            ).then_inc(s_g, 16)
        for j in range(TPP):
            nc.sync.wait_ge(s_g, 16 * (j + 1))
            nc.sync.dma_start(
                out[:, j * DIM : (j + 1) * DIM], g[:, j * DIM : (j + 1) * DIM]
            ).then_inc(s_out, 16)
    nc.compile()
    return nc


_NC_CACHE = None


def kernel(x: np.ndarray, weight: np.ndarray, **run_kwargs):
    global _NC_CACHE
    if _NC_CACHE is None:
        _NC_CACHE = build_nc()
    nc = _NC_CACHE

    x_flat = np.asarray(x).reshape(-1).astype(np.int32)
    w = np.ascontiguousarray(np.asarray(weight, dtype=np.float32))

    in_maps = [
        {
            "x": np.ascontiguousarray(x_flat[c * TPC : (c + 1) * TPC].reshape(P, TPP)),
            "weight": w,
        }
        for c in range(N_CORES)
    ]
    res = run_bass_kernel_spmd(nc, in_maps, core_ids=list(range(N_CORES)), **run_kwargs)
    # out [128, 1024] -> [1024, 128]: token p*TPP+j lives at [p, j*DIM:(j+1)*DIM]
    parts = [res.results[c]["out"].reshape(TPC, DIM) for c in range(N_CORES)]
    full = np.concatenate(parts, axis=0).reshape(B, S, DIM)
    if run_kwargs:
        return full, res
    return full
